# revision 3
# baseline (speedup 1.0000x reference)
"""Trainium2 Bass kernel for nn_Attention_21088289423660 (sparse_attention).

Reference computation (per token t = (b, n, m), feature dim D=256):
    kh = Wk^T k_t ; qh = Wq^T q_t ; v = Wv^T kh
    S  = kh - qh + pos_t
    attn = sigmoid(W2^T relu(W1^T S + b1) + b2)      (mask is all-ones)
    out  = Wo^T ((v + pos_t) * attn) + bo

Folded algebra (S is never materialized):
    h1  = A^T k + Bn^T q + W1^T pos + b1    A = Wk@W1, Bn = -Wq@W1
    v   = Wkv^T k                            Wkv = Wk@Wv
    h2  = W2^T relu(h1)
    attn = sigmoid(h2 + b2)
    out  = Wo^T ((v + pos) * attn)
The logit-path q and k terms run as fp8 e4m3 DoubleRow matmuls (q ships
as fp8 only; k ships bf16 for the value path plus an fp8 sidecar for
h1); everything else is bf16. 12 bf16 MMs + 2 DR MMs per 512-token
tile (rel_l2 ~1.2e-2 vs the 2e-2 budget).

Sharding: data-parallel over 8 cores; core c handles batch b=c//2 and
N-half (c%2) -> 16384 tokens/core, weights replicated.

Compute dtype: bf16 (PSUM accumulation fp32), device output bf16,
host adds bo and widens to fp32.
"""

import os
import sys

for _p in (
    "/root/.axon_site",
    "/root/.axon_site/_ro/trn_rl_repo",
    "/root/.axon_site/_ro/pypackages",
    "/opt/trn_rl_repo",
):
    if os.path.isdir(_p) and _p not in sys.path:
        sys.path.append(_p)

import numpy as np
import ml_dtypes
from contextlib import ExitStack

import concourse.bass as bass
import concourse.tile as tile
import concourse.bacc as bacc
from concourse import mybir
from concourse import bass_utils

BF16 = ml_dtypes.bfloat16

B, DIM, N, M = 4, 256, 4096, 8
NCORES = 8
NT = (B * N * M) // NCORES          # tokens per core = 16384
P = 128                              # partitions
NDC = DIM // P                       # d-chunks = 2
CHUNK = 2048                         # tokens per DMA chunk
FD = 512                             # tokens per matmul tile
F32 = mybir.dt.float32
BF = mybir.dt.bfloat16
F8 = mybir.dt.float8e4
FP8 = ml_dtypes.float8_e4m3

WARMUP_MMS = int(os.environ.get("KERNEL_WARMUP_MMS", "8"))

_CACHED_NC = None

# packed bf16 weight offsets (elements per partition row)
#   [0:512)    wh1: per c in {0,1}: [A_c (128) | W1_c (128)]
#   [512:768)  w2
#   [768:1792) wvo: per c: [Wkv_c (256) | Wo_c (256)]
WTOT = 1792


def _build_nc():
    """Build and compile the per-core Bass program (SPMD, identical on all cores).

    Software-pipelined at depth 4. Weights ride one chunky-descriptor DMA
    issued first on the sync ring (512B-descriptor weight loads starve
    behind bulk input traffic otherwise). Warm-up matmuls on garbage SBUF
    (no memset dependency) cover the DMA-fill window so the PE p-state is
    ramped when real matmuls start. Chunk 0/1 inputs are DMAed in
    512-token slices so tile j+1 never waits on a whole-chunk transfer.
    """
    nc = bacc.Bacc("TRN2", target_bir_lowering=False, debug=False)

    q8_d = nc.dram_tensor("q8s", (NDC, P, NT), F8, kind="ExternalInput").ap()
    k8_d = nc.dram_tensor("k8s", (NDC, P, NT), F8, kind="ExternalInput").ap()
    k_d = nc.dram_tensor("ks", (NDC, P, NT), BF, kind="ExternalInput").ap()
    pos_d = nc.dram_tensor("poss", (NDC, P, NT), BF, kind="ExternalInput").ap()
    wbf_d = nc.dram_tensor("wbf", (P, WTOT), BF, kind="ExternalInput").ap()
    # fp8 stationaries: per (p, c): [Bn8 (128) | A8 (128)]
    wf8_d = nc.dram_tensor("wf8", (P, NDC, 2 * P), F8, kind="ExternalInput").ap()
    bias_d = nc.dram_tensor("bias", (P, 3), F32, kind="ExternalInput").ap()
    out_d = nc.dram_tensor("out", (NDC, P, NT), BF, kind="ExternalOutput").ap()

    k_r = k_d.rearrange("c p t -> p c t")
    q8_r = q8_d.rearrange("c p t -> p c t")
    k8_r = k8_d.rearrange("c p t -> p c t")
    pos_r = pos_d.rearrange("c p t -> p c t")
    out_r = out_d.rearrange("c p t -> p c t")

    AF = mybir.ActivationFunctionType
    n_chunks = NT // CHUNK              # 8
    ipc = CHUNK // FD                   # iters per chunk = 4
    n_total = NT // FD                  # global iterations = 32

    with tile.TileContext(nc) as tc, ExitStack() as ctx:
        wpool = ctx.enter_context(tc.tile_pool(name="wpool", bufs=1))
        iopool = ctx.enter_context(tc.tile_pool(name="iopool", bufs=2))
        mid = ctx.enter_context(tc.tile_pool(name="mid", bufs=3))
        pp = ctx.enter_context(tc.tile_pool(name="pp", bufs=1, space="PSUM"))

        # --- weights first on the sync ring: chunky descriptors, tiny
        # total bytes, and everything downstream depends on them ---
        wbf_t = wpool.tile([P, WTOT], BF, tag="wbf", name="wbf")
        nc.sync.dma_start(wbf_t[:], wbf_d[:])
        wf8_t = wpool.tile([P, NDC, 2 * P], F8, tag="wf8", name="wf8")
        nc.sync.dma_start(wf8_t[:], wf8_d[:])

        # warm-up scratch: memset on DVE (its queue is empty at the head;
        # gpsimd takes ~6 us to reach its first instruction)
        wu_w = wpool.tile([P, P], BF, tag="wu_w", name="wu_w")
        nc.vector.memset(wu_w[:], 0.0)
        wu_in = wpool.tile([P, FD], BF, tag="wu_in", name="wu_in")
        nc.vector.memset(wu_in[:], 0.0)

        # first-chunk input tiles; all of chunk 0 arrives as 512-token
        # slices so s1(j) only waits for the data it actually reads
        kt0 = iopool.tile([P, NDC, CHUNK], BF, tag="kt", bufs=3, name="kt")
        qt0 = iopool.tile([P, NDC, CHUNK], F8, tag="qt", bufs=3, name="qt")
        k8t0 = iopool.tile([P, NDC, CHUNK], F8, tag="k8t", bufs=3, name="k8t")
        post0 = iopool.tile([P, NDC, CHUNK], BF, tag="post", bufs=3, name="post")
        sl0 = slice(0, FD)
        nc.sync.dma_start(kt0[:, :, sl0], k_r[:, :, sl0])
        nc.gpsimd.dma_start(qt0[:, :, sl0], q8_r[:, :, sl0])
        nc.gpsimd.dma_start(k8t0[:, :, sl0], k8_r[:, :, sl0])
        nc.scalar.dma_start(post0[:, :, sl0], pos_r[:, :, sl0])
        nc.scalar.dma_start(bias_d_tile := wpool.tile([P, 3], F32, tag="bias", name="bias"),
                            bias_d[:])
        bias_t = bias_d_tile

        # ACT spline tables next on the scalar queue (before later issues):
        # relu(0) needs them, but not before h1w(0) exists anyway
        dum_out = wpool.tile([P, 1], BF, tag="dum_out", name="dum_out")
        nc.scalar.activation(dum_out[:], wu_w[:, 0:1], AF.Relu)
        nc.scalar.activation(dum_out[:], wu_w[:, 0:1], AF.Sigmoid)

        def a_w(c):
            return wbf_t[:, 256 * c:256 * c + 128]

        def w1_w(c):
            return wbf_t[:, 256 * c + 128:256 * c + 256]

        def w2_w(e):
            return wbf_t[:, 512 + 128 * e:512 + 128 * (e + 1)]

        def wkv_w(c, e):
            return wbf_t[:, 768 + 512 * c + 128 * e:768 + 512 * c + 128 * (e + 1)]

        def wo_w(c, e):
            return wbf_t[:, 768 + 512 * c + 256 + 128 * e:768 + 512 * c + 256 + 128 * (e + 1)]

        def bn8_w():
            return wf8_t[:, :, 0:P]

        def a8_w():
            return wf8_t[:, :, P:2 * P]

        # --- HAM warm-up: one accumulation group of dummy matmuls (no
        # per-MM semaphores, back-to-back on the PE, no input deps) ---
        wu_ps = pp.tile([P, NDC, FD], F32, tag="xo", bufs=1, name="wu_ps")
        for i in range(WARMUP_MMS):
            nc.tensor.matmul(wu_ps[:, 0, :], wu_w[:], wu_in[:],
                             start=(i == 0), stop=(i == WARMUP_MMS - 1))

        io = {}        # chunk -> dict of io tiles
        st = {}        # global iter -> dict of stage tiles

        def load_chunk(ci):
            csl = bass.ts(ci, CHUNK)
            if ci == 0:
                # tiles + first slices loaded at the head; remainder here,
                # split per-512 so early tiles aren't gated on the tail
                kt, qt, k8t, post = kt0, qt0, k8t0, post0
                for s in range(1, ipc):
                    sl = bass.ts(s, FD)
                    nc.sync.dma_start(kt[:, :, sl], k_r[:, :, sl])
                    nc.gpsimd.dma_start(qt[:, :, sl], q8_r[:, :, sl])
                    nc.gpsimd.dma_start(k8t[:, :, sl], k8_r[:, :, sl])
                    nc.scalar.dma_start(post[:, :, sl], pos_r[:, :, sl])
                outt = iopool.tile([P, NDC, CHUNK], BF, tag="outt", bufs=2, name="outt")
                io[ci] = {"kt": kt, "qt": qt, "k8t": k8t, "post": post, "outt": outt}
                return
            kt = iopool.tile([P, NDC, CHUNK], BF, tag="kt", bufs=3, name="kt")
            qt = iopool.tile([P, NDC, CHUNK], F8, tag="qt", bufs=3, name="qt")
            k8t = iopool.tile([P, NDC, CHUNK], F8, tag="k8t", bufs=3, name="k8t")
            post = iopool.tile([P, NDC, CHUNK], BF, tag="post", bufs=3, name="post")
            if ci == 1:
                # still inside the fill window: per-1024 slices, pos on the
                # scalar ring to spread load
                for s in range(2):
                    sl = slice(s * 2 * FD, (s + 1) * 2 * FD)
                    gsl = slice(ci * CHUNK + s * 2 * FD, ci * CHUNK + (s + 1) * 2 * FD)
                    nc.sync.dma_start(kt[:, :, sl], k_r[:, :, gsl])
                    nc.gpsimd.dma_start(qt[:, :, sl], q8_r[:, :, gsl])
                    nc.gpsimd.dma_start(k8t[:, :, sl], k8_r[:, :, gsl])
                    nc.scalar.dma_start(post[:, :, sl], pos_r[:, :, gsl])
            else:
                nc.sync.dma_start(kt[:], k_r[:, :, csl])
                nc.sync.dma_start(post[:], pos_r[:, :, csl])
                nc.gpsimd.dma_start(qt[:], q8_r[:, :, csl])
                nc.gpsimd.dma_start(k8t[:], k8_r[:, :, csl])
            outt = iopool.tile([P, NDC, CHUNK], BF, tag="outt", bufs=2, name="outt")
            io[ci] = {"kt": kt, "qt": qt, "k8t": k8t, "post": post, "outt": outt}

        def s1(j):
            ci, it = divmod(j, ipc)
            t = io[ci]
            tsl = bass.ts(it, FD)
            h1w = pp.tile([P, FD], F32, tag="h1", bufs=2, name="h1w")
            # k and q logit terms: one fp8 DoubleRow matmul each (256
            # contraction in a 512-cycle slot)
            nc.tensor.matmul(h1w[:], a8_w(), t["k8t"][:, :, tsl],
                             start=True, stop=False,
                             perf_mode=mybir.MatmulPerfMode.DoubleRow)
            nc.tensor.matmul(h1w[:], bn8_w(), t["qt"][:, :, tsl],
                             start=False, stop=False,
                             perf_mode=mybir.MatmulPerfMode.DoubleRow)
            nc.tensor.matmul(h1w[:], w1_w(0), t["post"][:, 0, tsl],
                             start=False, stop=False)
            nc.tensor.matmul(h1w[:], w1_w(1), t["post"][:, 1, tsl],
                             start=False, stop=True)
            vw = pp.tile([P, NDC, FD], F32, tag="v", bufs=1, name="vw")
            for e in range(NDC):
                nc.tensor.matmul(vw[:, e, :], wkv_w(0, e), t["kt"][:, 0, tsl],
                                 start=True, stop=False)
                nc.tensor.matmul(vw[:, e, :], wkv_w(1, e), t["kt"][:, 1, tsl],
                                 start=False, stop=True)
            vp_t = mid.tile([P, NDC, FD], BF, tag="vp", bufs=4, name="vp_t")
            nc.vector.tensor_add(vp_t[:], vw[:], t["post"][:, :, tsl])
            st[j] = {"h1": h1w, "vp": vp_t}

        def s2(j):
            # relu -> h2 matmuls -> sigmoid (merged MLP stage)
            s = st[j]
            h1r = mid.tile([P, FD], BF, tag="h1r", bufs=2, name="h1r")
            nc.scalar.activation(h1r[:], s["h1"][:], AF.Relu, bias=bias_t[:, 0:1])
            h2w = pp.tile([P, NDC, FD], F32, tag="h2", bufs=1, name="h2w")
            for e in range(NDC):
                nc.tensor.matmul(h2w[:, e, :], w2_w(e), h1r[:],
                                 start=True, stop=True)
            at_t = mid.tile([P, NDC, FD], BF, tag="at", bufs=2, name="at_t")
            for e in range(NDC):
                nc.scalar.activation(at_t[:, e, :], h2w[:, e, :], AF.Sigmoid,
                                     bias=bias_t[:, 1 + e:2 + e])
            s["at"] = at_t

        def s4(j):
            s = st[j]
            g_t = mid.tile([P, NDC, FD], BF, tag="g", bufs=2, name="g_t")
            # drain tiles: DVE is ~2.3x faster per element than gpsimd and
            # idle there — shortens the end-of-pipeline latency chain
            e0_eng = nc.vector if j >= n_total - 3 else nc.gpsimd
            e0_eng.tensor_mul(g_t[:, 0, :], s["vp"][:, 0, :], s["at"][:, 0, :])
            nc.vector.tensor_mul(g_t[:, 1, :], s["vp"][:, 1, :], s["at"][:, 1, :])
            s["g"] = g_t

        def s5(j):
            ci, it = divmod(j, ipc)
            t = io[ci]
            tsl = bass.ts(it, FD)
            s = st[j]
            xo = pp.tile([P, NDC, FD], F32, tag="xo", bufs=1, name="xo")
            for e in range(NDC):
                nc.tensor.matmul(xo[:, e, :], wo_w(0, e), s["g"][:, 0, :],
                                 start=True, stop=False)
                nc.tensor.matmul(xo[:, e, :], wo_w(1, e), s["g"][:, 1, :],
                                 start=False, stop=True)
            nc.vector.tensor_copy(t["outt"][:, 0, tsl], xo[:, 0, :])
            if j >= n_total - 2:
                # drain: ACT still has sigmoids queued; DVE is idle and the
                # final out DMA waits on this copy
                nc.vector.tensor_copy(t["outt"][:, 1, tsl], xo[:, 1, :])
            else:
                nc.scalar.copy(t["outt"][:, 1, tsl], xo[:, 1, :])
            del st[j]
            # out DMA: per 1024 tokens; last chunk per 512-half so the final
            # transfer starts right after its own copy, shrinking the tail
            base = ci * CHUNK
            if ci == n_chunks - 1:
                gsl = slice(base + it * FD, base + (it + 1) * FD)
                nc.sync.dma_start(out_r[:, 0, gsl], t["outt"][:, 0, tsl])
                nc.sync.dma_start(out_r[:, 1, gsl], t["outt"][:, 1, tsl])
            elif it % 2 == 1:
                sl2 = slice((it - 1) * FD, (it + 1) * FD)
                nc.sync.dma_start(out_r[:, :, base + (it - 1) * FD:base + (it + 1) * FD],
                                  t["outt"][:, :, sl2])

        for t in range(n_total + 2):
            if t < n_total:
                if t % ipc == 0:
                    ci = t // ipc
                    if ci == 0:
                        load_chunk(0)
                        load_chunk(1)
                    elif ci + 1 < n_chunks:
                        load_chunk(ci + 1)
                s1(t)
            if t - 2 >= 0 and t - 2 < n_total:
                s4(t - 2)
            if t - 1 >= 0 and t - 1 < n_total:
                s2(t - 1)
            if t - 2 >= 0 and t - 2 < n_total:
                s5(t - 2)

    nc.compile()
    return nc


def _get_nc():
    global _CACHED_NC
    if _CACHED_NC is None:
        _CACHED_NC = _build_nc()
    return _CACHED_NC


def _prep_in_maps(q, k, pos, Wq, Wk, Wv, W1, b1, W2, b2, Wo, bo):
    q = np.asarray(q, dtype=np.float32)
    k = np.asarray(k, dtype=np.float32)
    pos = np.asarray(pos, dtype=np.float32)
    Wq32 = np.asarray(Wq, np.float32)
    Wk32 = np.asarray(Wk, np.float32)
    W132 = np.asarray(W1, np.float32)

    A = Wk32 @ W132                                   # (256, 128)
    Bn = -(Wq32 @ W132)                               # (256, 128)
    Wkv = (Wk32 @ np.asarray(Wv, np.float32)).astype(BF16)   # (256, 256)
    Wob = np.asarray(Wo, np.float32).astype(BF16)
    wvo = np.concatenate([Wkv, Wob], axis=1)          # (256, 512)

    # packed bf16 weights [P, WTOT]
    wh1 = np.concatenate([A.astype(BF16), W132.astype(BF16)], axis=1)  # (256, 256)
    wbf = np.empty((P, WTOT), dtype=BF16)
    for c in range(NDC):
        wbf[:, 256 * c:256 * (c + 1)] = wh1[c * P:(c + 1) * P, :]
        wbf[:, 768 + 512 * c:768 + 512 * (c + 1)] = wvo[c * P:(c + 1) * P, :]
    wbf[:, 512:768] = np.asarray(W2, np.float32).astype(BF16)

    # fp8 stationaries [P, NDC, 2P]: per (p, c): [Bn8 | A8]
    wf8 = np.empty((P, NDC, 2 * P), dtype=FP8)
    for c in range(NDC):
        wf8[:, c, 0:P] = Bn[c * P:(c + 1) * P, :].astype(FP8)
        wf8[:, c, P:2 * P] = A[c * P:(c + 1) * P, :].astype(FP8)

    bias = np.stack([np.asarray(b1, np.float32),
                     np.asarray(b2, np.float32)[:P],
                     np.asarray(b2, np.float32)[P:]], axis=1)  # (128, 3)

    weights = {
        "wbf": np.ascontiguousarray(wbf),
        "wf8": np.ascontiguousarray(wf8),
        "bias": np.ascontiguousarray(bias.astype(np.float32)),
    }

    nhalf = N // 2
    in_maps = []
    for c in range(NCORES):
        b = c // 2
        n0 = (c % 2) * nhalf
        qs = q[b, :, n0:n0 + nhalf, :].reshape(DIM, NT)
        ks = k[b, :, n0:n0 + nhalf, :].reshape(DIM, NT)
        ps = np.ascontiguousarray(
            pos[b, n0:n0 + nhalf].reshape(NT, DIM).T
        )
        m = dict(weights)
        m["q8s"] = qs.astype(FP8).reshape(NDC, P, NT)
        m["k8s"] = ks.astype(FP8).reshape(NDC, P, NT)
        m["ks"] = ks.astype(BF16).reshape(NDC, P, NT)
        m["poss"] = ps.astype(BF16).reshape(NDC, P, NT)
        in_maps.append(m)
    return in_maps


def _run(in_maps, trace=False, **kwargs):
    nc = _get_nc()
    return bass_utils.run_bass_kernel_spmd(
        nc, in_maps, core_ids=list(range(NCORES)), trace=trace, **kwargs
    )


def _assemble(results, bo, mask):
    bo = np.asarray(bo, np.float32)
    out = np.empty((B, DIM, N, M), dtype=np.float32)
    nhalf = N // 2
    for c in range(NCORES):
        b = c // 2
        n0 = (c % 2) * nhalf
        r = results[c]["out"].reshape(DIM, nhalf, M).astype(np.float32)
        r += bo[:, None, None]
        out[b, :, n0:n0 + nhalf, :] = r
    mask = np.asarray(mask)
    if not np.all(mask != 0):
        # masked positions: sigmoid(-1e9)=0 -> x=0 -> out = bo
        zb, zn, zm = np.nonzero(mask[..., 0] == 0)
        out[zb, :, zn, zm] = bo[None, :]
    return out


def kernel(q, k, pos, mask, Wq, Wk, Wv, W1, b1, W2, b2, Wo, bo):
    in_maps = _prep_in_maps(q, k, pos, Wq, Wk, Wv, W1, b1, W2, b2, Wo, bo)
    res = _run(in_maps)
    return _assemble(res.results, bo, mask)


def install_profile_hook():
    """Register the axon NTFF profiling hook (antenv.axon_hooks shim) so
    run_bass_kernel_spmd(trace=True) yields exec_time_ns + perfetto trace."""
    import types

    try:
        import antenv.axon_hooks  # noqa: F401
        return True
    except ImportError:
        pass
    try:
        from trn_agent_boot.trn_boot import _ntff_profile_via_ctypes
    except ImportError:
        return False
    hook = _ntff_profile_via_ctypes("/opt/axon/libaxon_pjrt.so")
    if hook is None:
        return False
    mod = types.ModuleType("antenv.axon_hooks")
    mod.get_axon_ntff_profile_hook = lambda: hook
    mod.set_axon_ntff_profile_hook = lambda h: None
    import antenv

    sys.modules["antenv.axon_hooks"] = mod
    antenv.axon_hooks = mod
    # artifact upload has no share reachable from this container
    bass_utils.upload_artifacts = lambda tmpdir: tmpdir
    return True


# revision 10
# speedup vs baseline: 1.0608x; 1.0608x over previous
"""Trainium2 Bass kernel for nn_Attention_21088289423660 (sparse_attention).

Reference computation (per token t = (b, n, m), feature dim D=256):
    kh = Wk^T k_t ; qh = Wq^T q_t ; v = Wv^T kh
    S  = kh - qh + pos_t
    attn = sigmoid(W2^T relu(W1^T S + b1) + b2)      (mask is all-ones)
    out  = Wo^T ((v + pos_t) * attn) + bo

Folded algebra (S is never materialized):
    h1  = A^T k + Bn^T q + W1^T pos + b1    A = Wk@W1, Bn = -Wq@W1
    v   = Wkv^T k                            Wkv = Wk@Wv
    h2  = W2^T relu(h1)
    attn = sigmoid(h2 + b2)
    out  = Wo^T ((v + pos) * attn)
The logit-path q and k terms run as fp8 e4m3 DoubleRow matmuls (q ships
as fp8 only; k ships bf16 for the value path plus an fp8 sidecar for
h1); everything else is bf16. 12 bf16 MMs + 2 DR MMs per 512-token
tile (rel_l2 ~1.2e-2 vs the 2e-2 budget).

Sharding: data-parallel over 8 cores; core c handles batch b=c//2 and
N-half (c%2) -> 16384 tokens/core, weights replicated.

Compute dtype: bf16 (PSUM accumulation fp32), device output bf16,
host adds bo and widens to fp32.
"""

import os
import sys

for _p in (
    "/root/.axon_site",
    "/root/.axon_site/_ro/trn_rl_repo",
    "/root/.axon_site/_ro/pypackages",
    "/opt/trn_rl_repo",
):
    if os.path.isdir(_p) and _p not in sys.path:
        sys.path.append(_p)

import numpy as np
import ml_dtypes
from contextlib import ExitStack

import concourse.bass as bass
import concourse.tile as tile
import concourse.bacc as bacc
from concourse import mybir
from concourse import bass_utils

BF16 = ml_dtypes.bfloat16

B, DIM, N, M = 4, 256, 4096, 8
NCORES = 8
NT = (B * N * M) // NCORES          # tokens per core = 16384
P = 128                              # partitions
NDC = DIM // P                       # d-chunks = 2
CHUNK = 2048                         # tokens per DMA chunk
FD = 512                             # tokens per matmul tile
F32 = mybir.dt.float32
BF = mybir.dt.bfloat16
F8 = mybir.dt.float8e4
FP8 = ml_dtypes.float8_e4m3

WARMUP_MMS = int(os.environ.get("KERNEL_WARMUP_MMS", "10"))

_CACHED_NC = None

# packed bf16 weight offsets (elements per partition row)
#   [0:512)    wh1: per c in {0,1}: [A_c (128) | W1_c (128)]
#   [512:768)  w2
#   [768:1792) wvo: per c: [Wkv_c (256) | Wo_c (256)]
WTOT = 1792


def _build_nc():
    """Build and compile the per-core Bass program (SPMD, identical on all cores).

    Software-pipelined at depth 4. Weights ride one chunky-descriptor DMA
    issued first on the sync ring (512B-descriptor weight loads starve
    behind bulk input traffic otherwise). Warm-up matmuls on garbage SBUF
    (no memset dependency) cover the DMA-fill window so the PE p-state is
    ramped when real matmuls start. Chunk 0/1 inputs are DMAed in
    512-token slices so tile j+1 never waits on a whole-chunk transfer.
    """
    nc = bacc.Bacc("TRN2", target_bir_lowering=False, debug=False)

    # packed token streams: one DMA issue moves both members of a pair
    # (s=0/1), halving engine-queue issue pressure
    qk8_d = nc.dram_tensor("qk8s", (NDC, 2, P, NT), F8, kind="ExternalInput").ap()
    kp_d = nc.dram_tensor("kps", (NDC, 2, P, NT), BF, kind="ExternalInput").ap()
    wbf_d = nc.dram_tensor("wbf", (P, WTOT), BF, kind="ExternalInput").ap()
    # fp8 stationaries: per (p, c): [Bn8 (128) | A8 (128)]
    wf8_d = nc.dram_tensor("wf8", (P, NDC, 2 * P), F8, kind="ExternalInput").ap()
    bias_d = nc.dram_tensor("bias", (P, 3), F32, kind="ExternalInput").ap()
    out_d = nc.dram_tensor("out", (NDC, P, NT), BF, kind="ExternalOutput").ap()

    qk8_r = qk8_d.rearrange("c s p t -> p c s t")
    kp_r = kp_d.rearrange("c s p t -> p c s t")
    out_r = out_d.rearrange("c p t -> p c t")

    AF = mybir.ActivationFunctionType
    n_chunks = NT // CHUNK              # 8
    ipc = CHUNK // FD                   # iters per chunk = 4
    n_total = NT // FD                  # global iterations = 32

    with tile.TileContext(nc) as tc, ExitStack() as ctx:
        wpool = ctx.enter_context(tc.tile_pool(name="wpool", bufs=1))
        iopool = ctx.enter_context(tc.tile_pool(name="iopool", bufs=2))
        mid = ctx.enter_context(tc.tile_pool(name="mid", bufs=3))
        pp = ctx.enter_context(tc.tile_pool(name="pp", bufs=1, space="PSUM"))

        # --- weights first on the sync ring: chunky descriptors, tiny
        # total bytes, and everything downstream depends on them ---
        wbf_t = wpool.tile([P, WTOT], BF, tag="wbf", name="wbf")
        nc.sync.dma_start(wbf_t[:], wbf_d[:])
        wf8_t = wpool.tile([P, NDC, 2 * P], F8, tag="wf8", name="wf8")
        nc.sync.dma_start(wf8_t[:], wf8_d[:])

        # warm-up scratch: memset on DVE (its queue is empty at the head;
        # gpsimd takes ~6 us to reach its first instruction)
        wu_w = wpool.tile([P, P], BF, tag="wu_w", name="wu_w")
        nc.vector.memset(wu_w[:], 0.0)
        wu_in = wpool.tile([P, FD], BF, tag="wu_in", name="wu_in")
        nc.vector.memset(wu_in[:], 0.0)

        # first-chunk input tiles; all of chunk 0 arrives as 512-token
        # slices so s1(j) only waits for the data it actually reads
        kp0 = iopool.tile([P, NDC, 2, CHUNK], BF, tag="kp", bufs=3, name="kp")
        qk0 = iopool.tile([P, NDC, 2, CHUNK], F8, tag="qk", bufs=3, name="qk")
        sl0 = slice(0, FD)
        nc.sync.dma_start(kp0[:, :, :, sl0], kp_r[:, :, :, sl0])
        nc.gpsimd.dma_start(qk0[:, :, :, sl0], qk8_r[:, :, :, sl0])
        nc.scalar.dma_start(bias_d_tile := wpool.tile([P, 3], F32, tag="bias", name="bias"),
                            bias_d[:])
        bias_t = bias_d_tile

        # ACT spline tables next on the scalar queue (before later issues):
        # relu(0) needs them, but not before h1w(0) exists anyway
        dum_out = wpool.tile([P, 1], BF, tag="dum_out", name="dum_out")
        nc.scalar.activation(dum_out[:], wu_w[:, 0:1], AF.Relu)
        nc.scalar.activation(dum_out[:], wu_w[:, 0:1], AF.Sigmoid)

        def a_w(c):
            return wbf_t[:, 256 * c:256 * c + 128]

        def w1_w(c):
            return wbf_t[:, 256 * c + 128:256 * c + 256]

        def w2_w(e):
            return wbf_t[:, 512 + 128 * e:512 + 128 * (e + 1)]

        def wkv_w(c, e):
            return wbf_t[:, 768 + 512 * c + 128 * e:768 + 512 * c + 128 * (e + 1)]

        def wo_w(c, e):
            return wbf_t[:, 768 + 512 * c + 256 + 128 * e:768 + 512 * c + 256 + 128 * (e + 1)]

        def bn8_w():
            return wf8_t[:, :, 0:P]

        def a8_w():
            return wf8_t[:, :, P:2 * P]

        # --- HAM warm-up: one accumulation group of dummy matmuls (no
        # per-MM semaphores, back-to-back on the PE, no input deps) ---
        wu_ps = pp.tile([P, NDC, FD], F32, tag="xo", bufs=1, name="wu_ps")
        for i in range(WARMUP_MMS):
            nc.tensor.matmul(wu_ps[:, 0, :], wu_w[:], wu_in[:],
                             start=(i == 0), stop=(i == WARMUP_MMS - 1))

        io = {}        # chunk -> dict of io tiles
        st = {}        # global iter -> dict of stage tiles

        def load_chunk(ci):
            csl = bass.ts(ci, CHUNK)
            if ci == 0:
                # tiles + first slices loaded at the head; remainder here,
                # split per-512 so early tiles aren't gated on the tail
                kp, qk = kp0, qk0
                for s in range(1, ipc):
                    sl = bass.ts(s, FD)
                    nc.sync.dma_start(kp[:, :, :, sl], kp_r[:, :, :, sl])
                    nc.gpsimd.dma_start(qk[:, :, :, sl], qk8_r[:, :, :, sl])
                outt = iopool.tile([P, NDC, CHUNK], BF, tag="outt", bufs=2, name="outt")
                io[ci] = {"kp": kp, "qk": qk, "outt": outt}
                return
            kp = iopool.tile([P, NDC, 2, CHUNK], BF, tag="kp", bufs=3, name="kp")
            qk = iopool.tile([P, NDC, 2, CHUNK], F8, tag="qk", bufs=3, name="qk")
            if ci == 1:
                # still inside the fill window: per-1024 slices
                for s in range(2):
                    sl = slice(s * 2 * FD, (s + 1) * 2 * FD)
                    gsl = slice(ci * CHUNK + s * 2 * FD, ci * CHUNK + (s + 1) * 2 * FD)
                    nc.sync.dma_start(kp[:, :, :, sl], kp_r[:, :, :, gsl])
                    nc.gpsimd.dma_start(qk[:, :, :, sl], qk8_r[:, :, :, gsl])
            else:
                nc.sync.dma_start(kp[:], kp_r[:, :, :, csl])
                nc.gpsimd.dma_start(qk[:], qk8_r[:, :, :, csl])
            outt = iopool.tile([P, NDC, CHUNK], BF, tag="outt", bufs=2, name="outt")
            io[ci] = {"kp": kp, "qk": qk, "outt": outt}

        def s1(j):
            ci, it = divmod(j, ipc)
            t = io[ci]
            tsl = bass.ts(it, FD)
            h1w = pp.tile([P, FD], F32, tag="h1", bufs=2, name="h1w")
            # k and q logit terms: one fp8 DoubleRow matmul each (256
            # contraction in a 512-cycle slot); bf16 MMs interleaved so a
            # DR LDWEIGHTS never has to hide behind another DR matmul
            nc.tensor.matmul(h1w[:], a8_w(), t["qk"][:, :, 0, tsl],
                             start=True, stop=False,
                             perf_mode=mybir.MatmulPerfMode.DoubleRow)
            nc.tensor.matmul(h1w[:], w1_w(0), t["kp"][:, 0, 1, tsl],
                             start=False, stop=False)
            nc.tensor.matmul(h1w[:], bn8_w(), t["qk"][:, :, 1, tsl],
                             start=False, stop=False,
                             perf_mode=mybir.MatmulPerfMode.DoubleRow)
            nc.tensor.matmul(h1w[:], w1_w(1), t["kp"][:, 1, 1, tsl],
                             start=False, stop=True)
            vw = pp.tile([P, NDC, FD], F32, tag="v", bufs=1, name="vw")
            for e in range(NDC):
                nc.tensor.matmul(vw[:, e, :], wkv_w(0, e), t["kp"][:, 0, 0, tsl],
                                 start=True, stop=False)
                nc.tensor.matmul(vw[:, e, :], wkv_w(1, e), t["kp"][:, 1, 0, tsl],
                                 start=False, stop=True)
            vp_t = mid.tile([P, NDC, FD], BF, tag="vp", bufs=4, name="vp_t")
            nc.vector.tensor_add(vp_t[:], vw[:], t["kp"][:, :, 1, tsl])
            st[j] = {"h1": h1w, "vp": vp_t}

        def s2(j):
            # relu -> h2 matmuls -> sigmoid (merged MLP stage)
            s = st[j]
            h1r = mid.tile([P, FD], BF, tag="h1r", bufs=2, name="h1r")
            nc.scalar.activation(h1r[:], s["h1"][:], AF.Relu, bias=bias_t[:, 0:1])
            h2w = pp.tile([P, NDC, FD], F32, tag="h2", bufs=1, name="h2w")
            for e in range(NDC):
                nc.tensor.matmul(h2w[:, e, :], w2_w(e), h1r[:],
                                 start=True, stop=True)
            at_t = mid.tile([P, NDC, FD], BF, tag="at", bufs=2, name="at_t")
            for e in range(NDC):
                nc.scalar.activation(at_t[:, e, :], h2w[:, e, :], AF.Sigmoid,
                                     bias=bias_t[:, 1 + e:2 + e])
            s["at"] = at_t

        def s4(j):
            s = st[j]
            g_t = mid.tile([P, NDC, FD], BF, tag="g", bufs=2, name="g_t")
            # drain/fill tiles: DVE is ~2.3x faster per element than gpsimd
            # and idle there; gpsimd is also busy issuing head DMAs early
            e0_eng = nc.vector if (j >= n_total - 3 or j < 6) else nc.gpsimd
            e0_eng.tensor_mul(g_t[:, 0, :], s["vp"][:, 0, :], s["at"][:, 0, :])
            nc.vector.tensor_mul(g_t[:, 1, :], s["vp"][:, 1, :], s["at"][:, 1, :])
            s["g"] = g_t

        def s5(j):
            ci, it = divmod(j, ipc)
            t = io[ci]
            tsl = bass.ts(it, FD)
            s = st[j]
            xo = pp.tile([P, NDC, FD], F32, tag="xo", bufs=1, name="xo")
            for e in range(NDC):
                nc.tensor.matmul(xo[:, e, :], wo_w(0, e), s["g"][:, 0, :],
                                 start=True, stop=False)
                nc.tensor.matmul(xo[:, e, :], wo_w(1, e), s["g"][:, 1, :],
                                 start=False, stop=True)
            nc.vector.tensor_copy(t["outt"][:, 0, tsl], xo[:, 0, :])
            if j >= n_total - 2:
                # drain: ACT still has sigmoids queued; DVE is idle and the
                # final out DMA waits on this copy
                nc.vector.tensor_copy(t["outt"][:, 1, tsl], xo[:, 1, :])
            else:
                nc.scalar.copy(t["outt"][:, 1, tsl], xo[:, 1, :])
            del st[j]
            # out DMA: per 1024 tokens; last chunk per 512-half so the final
            # transfer starts right after its own copy, shrinking the tail
            base = ci * CHUNK
            if ci == n_chunks - 1:
                gsl = slice(base + it * FD, base + (it + 1) * FD)
                nc.sync.dma_start(out_r[:, 0, gsl], t["outt"][:, 0, tsl])
                nc.sync.dma_start(out_r[:, 1, gsl], t["outt"][:, 1, tsl])
            elif it % 2 == 1:
                sl2 = slice((it - 1) * FD, (it + 1) * FD)
                nc.sync.dma_start(out_r[:, :, base + (it - 1) * FD:base + (it + 1) * FD],
                                  t["outt"][:, :, sl2])

        for t in range(n_total + 2):
            if t < n_total:
                if t % ipc == 0:
                    ci = t // ipc
                    if ci == 0:
                        load_chunk(0)
                        load_chunk(1)
                    elif ci + 1 < n_chunks:
                        load_chunk(ci + 1)
                s1(t)
            if t - 2 >= 0 and t - 2 < n_total:
                s4(t - 2)
            if t - 1 >= 0 and t - 1 < n_total:
                s2(t - 1)
            if t - 2 >= 0 and t - 2 < n_total:
                s5(t - 2)

    nc.compile()
    return nc


def _get_nc():
    global _CACHED_NC
    if _CACHED_NC is None:
        _CACHED_NC = _build_nc()
    return _CACHED_NC


def _prep_in_maps(q, k, pos, Wq, Wk, Wv, W1, b1, W2, b2, Wo, bo):
    q = np.asarray(q, dtype=np.float32)
    k = np.asarray(k, dtype=np.float32)
    pos = np.asarray(pos, dtype=np.float32)
    Wq32 = np.asarray(Wq, np.float32)
    Wk32 = np.asarray(Wk, np.float32)
    W132 = np.asarray(W1, np.float32)

    A = Wk32 @ W132                                   # (256, 128)
    Bn = -(Wq32 @ W132)                               # (256, 128)
    Wkv = (Wk32 @ np.asarray(Wv, np.float32)).astype(BF16)   # (256, 256)
    Wob = np.asarray(Wo, np.float32).astype(BF16)
    wvo = np.concatenate([Wkv, Wob], axis=1)          # (256, 512)

    # packed bf16 weights [P, WTOT]
    wh1 = np.concatenate([A.astype(BF16), W132.astype(BF16)], axis=1)  # (256, 256)
    wbf = np.empty((P, WTOT), dtype=BF16)
    for c in range(NDC):
        wbf[:, 256 * c:256 * (c + 1)] = wh1[c * P:(c + 1) * P, :]
        wbf[:, 768 + 512 * c:768 + 512 * (c + 1)] = wvo[c * P:(c + 1) * P, :]
    wbf[:, 512:768] = np.asarray(W2, np.float32).astype(BF16)

    # fp8 stationaries [P, NDC, 2P]: per (p, c): [Bn8 | A8]
    wf8 = np.empty((P, NDC, 2 * P), dtype=FP8)
    for c in range(NDC):
        wf8[:, c, 0:P] = Bn[c * P:(c + 1) * P, :].astype(FP8)
        wf8[:, c, P:2 * P] = A[c * P:(c + 1) * P, :].astype(FP8)

    bias = np.stack([np.asarray(b1, np.float32),
                     np.asarray(b2, np.float32)[:P],
                     np.asarray(b2, np.float32)[P:]], axis=1)  # (128, 3)

    weights = {
        "wbf": np.ascontiguousarray(wbf),
        "wf8": np.ascontiguousarray(wf8),
        "bias": np.ascontiguousarray(bias.astype(np.float32)),
    }

    nhalf = N // 2
    in_maps = []
    for c in range(NCORES):
        b = c // 2
        n0 = (c % 2) * nhalf
        qs = q[b, :, n0:n0 + nhalf, :].reshape(DIM, NT)
        ks = k[b, :, n0:n0 + nhalf, :].reshape(DIM, NT)
        ps = np.ascontiguousarray(
            pos[b, n0:n0 + nhalf].reshape(NT, DIM).T
        )
        m = dict(weights)
        # paired streams: qk8[c, 0] = k8, qk8[c, 1] = q8;
        #                 kp[c, 0] = k (bf16), kp[c, 1] = pos (bf16)
        qk8 = np.empty((NDC, 2, P, NT), dtype=FP8)
        qk8[:, 0] = ks.astype(FP8).reshape(NDC, P, NT)
        qk8[:, 1] = qs.astype(FP8).reshape(NDC, P, NT)
        kp = np.empty((NDC, 2, P, NT), dtype=BF16)
        kp[:, 0] = ks.astype(BF16).reshape(NDC, P, NT)
        kp[:, 1] = ps.astype(BF16).reshape(NDC, P, NT)
        m["qk8s"] = qk8
        m["kps"] = kp
        in_maps.append(m)
    return in_maps


def _run(in_maps, trace=False, **kwargs):
    nc = _get_nc()
    return bass_utils.run_bass_kernel_spmd(
        nc, in_maps, core_ids=list(range(NCORES)), trace=trace, **kwargs
    )


def _assemble(results, bo, mask):
    bo = np.asarray(bo, np.float32)
    out = np.empty((B, DIM, N, M), dtype=np.float32)
    nhalf = N // 2
    for c in range(NCORES):
        b = c // 2
        n0 = (c % 2) * nhalf
        r = results[c]["out"].reshape(DIM, nhalf, M).astype(np.float32)
        r += bo[:, None, None]
        out[b, :, n0:n0 + nhalf, :] = r
    mask = np.asarray(mask)
    if not np.all(mask != 0):
        # masked positions: sigmoid(-1e9)=0 -> x=0 -> out = bo
        zb, zn, zm = np.nonzero(mask[..., 0] == 0)
        out[zb, :, zn, zm] = bo[None, :]
    return out


def kernel(q, k, pos, mask, Wq, Wk, Wv, W1, b1, W2, b2, Wo, bo):
    in_maps = _prep_in_maps(q, k, pos, Wq, Wk, Wv, W1, b1, W2, b2, Wo, bo)
    res = _run(in_maps)
    return _assemble(res.results, bo, mask)


def install_profile_hook():
    """Register the axon NTFF profiling hook (antenv.axon_hooks shim) so
    run_bass_kernel_spmd(trace=True) yields exec_time_ns + perfetto trace."""
    import types

    try:
        import antenv.axon_hooks  # noqa: F401
        return True
    except ImportError:
        pass
    try:
        from trn_agent_boot.trn_boot import _ntff_profile_via_ctypes
    except ImportError:
        return False
    hook = _ntff_profile_via_ctypes("/opt/axon/libaxon_pjrt.so")
    if hook is None:
        return False
    mod = types.ModuleType("antenv.axon_hooks")
    mod.get_axon_ntff_profile_hook = lambda: hook
    mod.set_axon_ntff_profile_hook = lambda h: None
    import antenv

    sys.modules["antenv.axon_hooks"] = mod
    antenv.axon_hooks = mod
    # artifact upload has no share reachable from this container
    bass_utils.upload_artifacts = lambda tmpdir: tmpdir
    return True
